# revision 1
# baseline (speedup 1.0000x reference)
"""MeshMeanFlowNet block on 8 Trainium2 NeuronCores.

Sharding: data-parallel over B (one batch element per core), no collectives.
All activations are kept feature-major on device ([feature, token]) so every
linear layer consumes its input directly as the matmul moving operand and
produces feature-major output. The attention softmax is computed in the
transposed layout S^T[j, i] (j = key token on partitions, i = query token on
the free axis); the softmax denominator comes for free from a ones-row
appended to V, so no on-device transposes are needed anywhere: V is produced
token-major by swapping the matmul operand roles for the v-part of the qkv
projection.

The per-edge-type/per-head bias never materializes a gather: with only 4 edge
types, softmax shift-invariance removes type 3, and the remaining three
(table[e,h] - table[3,h]) deltas are applied as three fused
(mask_e * c_eh + acc) scalar_tensor_tensor ops on top of the score PSUM,
where mask_e = (edge == e) is shared across all heads.
"""

import sys

sys.path.insert(0, "/opt/trn_rl_repo")

import ml_dtypes
import numpy as np

B, V, D, H = 8, 1024, 512, 8
HD = D // H  # 64
NCORES = 8

_cache = {}


def _build_program(cb, probe=False):
    """cb: [3][H] float bias deltas (edge_table[e,h] - edge_table[3,h])."""
    import contextlib

    import concourse.bacc as bacc
    import concourse.tile as tile
    from concourse import mybir

    f32 = mybir.dt.float32
    f32r = mybir.dt.float32r
    bf16 = mybir.dt.bfloat16
    ALU = mybir.AluOpType
    ACTF = mybir.ActivationFunctionType

    nc = bacc.Bacc("TRN2", target_bir_lowering=False, debug=False,
                   num_devices=NCORES)

    # ---- DRAM I/O (per-core shard, host pre-laid-out) ----
    xT = nc.dram_tensor("xT", [D, V], f32r, kind="ExternalInput")
    eiT = nc.dram_tensor("eiT", [V, V], bf16, kind="ExternalInput")  # [j, i]
    condc = nc.dram_tensor("condc", [4, 128], f32, kind="ExternalInput")
    wqk = nc.dram_tensor("wqk", [D, 1024], f32r, kind="ExternalInput")
    wv = nc.dram_tensor("wv", [D, 512], f32r, kind="ExternalInput")
    wada = nc.dram_tensor("wada", [D, 2048], f32r, kind="ExternalInput")
    bada = nc.dram_tensor("bada", [16, 128], f32, kind="ExternalInput")
    wproj = nc.dram_tensor("wproj", [D, D], f32r, kind="ExternalInput")
    bproj = nc.dram_tensor("bproj", [4, 128], f32, kind="ExternalInput")
    wm1 = nc.dram_tensor("wm1", [D, 2048], f32r, kind="ExternalInput")
    bm1 = nc.dram_tensor("bm1", [16, 128], f32, kind="ExternalInput")
    wm2 = nc.dram_tensor("wm2", [2048, D], f32r, kind="ExternalInput")
    bm2 = nc.dram_tensor("bm2", [4, 128], f32, kind="ExternalInput")
    onesc = nc.dram_tensor("onesc", [128, 8], f32r, kind="ExternalInput")
    onesb = nc.dram_tensor("onesb", [128, 8], bf16, kind="ExternalInput")
    yT = nc.dram_tensor("yT", [D, V], f32, kind="ExternalOutput")
    if probe:
        p_params = nc.dram_tensor("p_params", [128, 16], f32, kind="ExternalOutput")
        p_h1 = nc.dram_tensor("p_h1", [D, V], f32, kind="ExternalOutput")
        p_qk = nc.dram_tensor("p_qk", [8, 128, V], f32, kind="ExternalOutput")
        p_vaug = nc.dram_tensor("p_vaug", [8, 128, 8, 65], f32, kind="ExternalOutput")
        p_att = nc.dram_tensor("p_att", [D, V], f32, kind="ExternalOutput")
        p_x2 = nc.dram_tensor("p_x2", [D, V], f32, kind="ExternalOutput")
        p_h2 = nc.dram_tensor("p_h2", [D, V], f32, kind="ExternalOutput")
        p_L = nc.dram_tensor("p_L", [128, 512], bf16, kind="ExternalOutput")
        p_P = nc.dram_tensor("p_P", [128, 512], bf16, kind="ExternalOutput")
        p_m0 = nc.dram_tensor("p_m0", [128, 512], bf16, kind="ExternalOutput")
        p_ops = nc.dram_tensor("p_ops", [65, 512], f32, kind="ExternalOutput")

    def mm(out, lhsT, rhs, **kw):
        nc.tensor.matmul(out, lhsT.bitcast(f32r), rhs.bitcast(f32r), **kw)

    with tile.TileContext(nc) as tc:
        with contextlib.ExitStack() as ctx:
            # ---- whole-program pool: ~60KB/partition ----
            persist = ctx.enter_context(tc.tile_pool(name="persist", bufs=1))

            ones = persist.tile([128, 1], f32r, tag="ones")
            nc.sync.dma_start(out=ones, in_=onesc[:, 0:1])
            epst = persist.tile([1, 1], f32, tag="eps")
            nc.vector.memset(epst, 1e-5)

            # x (feature-major)
            xT_t = [persist.tile([128, V], f32r, tag=f"xT{kc}",
                                 name=f"xT_t{kc}") for kc in range(4)]
            for kc in range(4):
                nc.sync.dma_start(out=xT_t[kc],
                                  in_=xT[kc * 128:(kc + 1) * 128, :])

            bada_t = persist.tile([128, 16], f32, tag="bada")
            nc.sync.dma_start(out=bada_t, in_=bada[:].rearrange("c p -> p c"))
            bp_t = persist.tile([128, 4], f32, tag="bproj")
            nc.sync.dma_start(out=bp_t, in_=bproj[:].rearrange("c p -> p c"))
            bm1_t = persist.tile([128, 16], f32, tag="bm1")
            nc.sync.dma_start(out=bm1_t, in_=bm1[:].rearrange("c p -> p c"))
            bm2_t = persist.tile([128, 4], f32, tag="bm2")
            nc.sync.dma_start(out=bm2_t, in_=bm2[:].rearrange("c p -> p c"))

            x2 = [persist.tile([128, V], f32r, tag=f"x2_{kc}",
                                name=f"x2_{kc}") for kc in range(4)]
            params = persist.tile([128, 16], f32, tag="params")
            nparams = persist.tile([128, 8], f32, tag="nparams")

            # ---------- AdaLN parameter path ----------
            with tc.tile_pool(name="adaw", bufs=1) as adaw, \
                    tc.tile_pool(name="adap", bufs=2, space="PSUM") as adap:
                condt = adaw.tile([128, 5], f32, tag="cond")
                nc.sync.dma_start(out=condt[:, 0:4],
                                  in_=condc[:].rearrange("c p -> p c"))
                nc.vector.memset(condt[:, 4:5], 0.0)
                scond = adaw.tile([128, 5], f32r, tag="scond")
                nc.scalar.activation(scond, condt, ACTF.Silu)
                wada_t = [adaw.tile([128, 2048], f32r, tag=f"wada{kc}",
                                    name="wada_t") for kc in range(4)]
                for kc in range(4):
                    nc.sync.dma_start(out=wada_t[kc],
                                      in_=wada[kc * 128:(kc + 1) * 128, :])
                pp = adap.tile([2, 2048], f32, tag="pada")
                for oc in range(4):
                    s = slice(oc * 512, oc * 512 + 512)
                    for kc in range(4):
                        mm(pp[:, s], scond[:, kc:kc + 2], wada_t[kc][:, s],
                           start=(kc == 0), stop=(kc == 3))
                prow = adaw.tile([1, 2048], f32, tag="prow")
                nc.scalar.copy(prow, pp[0:1, :])
                pscat = adaw.tile([128, 16], f32, tag="pscat")
                for md in range(16):
                    nc.sync.dma_start(
                        out=pscat[:, md:md + 1],
                        in_=prow[0:1, md * 128:(md + 1) * 128])
                nc.vector.tensor_add(params, pscat, bada_t)
                for ln in range(2):
                    for kc in range(4):
                        sc = ln * 8 + kc
                        nc.vector.tensor_scalar(
                            nparams[:, ln * 4 + kc:ln * 4 + kc + 1],
                            params[:, sc:sc + 1], -1.0, None, ALU.mult)
                if probe:
                    nc.sync.dma_start(out=p_params[:], in_=params)

            def adaln(src_tiles, ln_idx, dst_pool, out_tag):
                """LayerNorm over the partition (feature) axis + adaptive
                affine from `params`. Returns 4 feature-major tiles."""
                out = [dst_pool.tile([128, V], f32r, tag=f"{out_tag}{kc}",
                                     name=f"ln_{out_tag}{kc}")
                       for kc in range(4)]
                with tc.tile_pool(name="lnt", bufs=1) as lnt, \
                        tc.tile_pool(name="lnp", bufs=1,
                                     space="PSUM") as lnp:
                    ps_s = lnp.tile([1, V], f32, tag="lnsum")
                    ps_q = lnp.tile([1, V], f32, tag="lnsqsum")
                    for kc in range(4):
                        sq = lnt.tile([128, V], f32r, tag="lnsq", bufs=2,
                                      name="sq")
                        nc.scalar.square(sq, src_tiles[kc].bitcast(f32))
                        for nh in range(2):
                            s = slice(nh * 512, nh * 512 + 512)
                            mm(ps_s[:, s], ones, src_tiles[kc][:, s],
                               start=(kc == 0), stop=(kc == 3))
                            mm(ps_q[:, s], ones, sq[:, s],
                               start=(kc == 0), stop=(kc == 3))
                    mean = lnt.tile([1, V], f32, tag="mean")
                    nc.scalar.mul(mean, ps_s, 1.0 / D)
                    msq = lnt.tile([1, V], f32, tag="msq")
                    nc.vector.tensor_mul(msq, mean, mean)
                    std = lnt.tile([1, V], f32, tag="std")
                    nc.vector.scalar_tensor_tensor(std, ps_q, 1.0 / D, msq,
                                                   ALU.mult, ALU.subtract)
                    nc.scalar.activation(std, std, ACTF.Sqrt, bias=epst)
                    sT = lnt.tile([128, 8], f32, tag="sT")
                    for c in range(8):
                        nc.sync.dma_start(out=sT[:, c:c + 1],
                                          in_=std[0:1,
                                                  c * 128:(c + 1) * 128])
                    rT = lnt.tile([128, 8], f32, tag="rT")
                    nc.vector.reciprocal(rT, sT)
                    r = lnt.tile([1, V], f32, tag="r")
                    for c in range(8):
                        nc.sync.dma_start(out=r[0:1,
                                              c * 128:(c + 1) * 128],
                                          in_=rT[:, c:c + 1])
                    mr = lnt.tile([1, V], f32, tag="mr")
                    nc.vector.tensor_mul(mr, mean, r)
                    rb = lnt.tile([128, V], f32, tag="rb")
                    nc.gpsimd.partition_broadcast(rb, r)
                    mrb = lnt.tile([128, V], f32, tag="mrb")
                    nc.gpsimd.partition_broadcast(mrb, mr)
                    for kc in range(4):
                        smd = ln_idx * 8 + kc
                        tmd = ln_idx * 8 + 4 + kc
                        u = lnt.tile([128, V], f32, tag="lnu", bufs=2,
                                     name="u")
                        nc.vector.tensor_mul(u, src_tiles[kc].bitcast(f32),
                                             rb)
                        u2 = lnt.tile([128, V], f32, tag="lnu2", bufs=2,
                                      name="u2")
                        nc.vector.scalar_tensor_tensor(
                            u2, mrb, -1.0, u, ALU.mult, ALU.add)
                        nc.vector.tensor_scalar(out[kc], u2,
                                                params[:, smd:smd + 1],
                                                params[:, tmd:tmd + 1],
                                                ALU.mult, ALU.add)
                return out

            # ---- attention-lifetime pool: qk 32K + vaug 16.25K + att 16K ----
            with tc.tile_pool(name="attlife", bufs=1) as attlife:
                qk = [attlife.tile([128, V], bf16, tag=f"qk{m}",
                                   name=f"qk{m}") for m in range(8)]
                vaug = [attlife.tile([128, 8, 65], bf16, tag=f"vaug{t}",
                                     name=f"vaug{t}") for t in range(8)]
                att = [attlife.tile([128, V], f32r, tag=f"att{kc}",
                                    name=f"att{kc}") for kc in range(4)]

                # h1 = AdaLN1(x); qk feature-major; v token-major
                with tc.tile_pool(name="h1pool", bufs=1) as h1pool:
                    h1 = adaln(xT_t, 0, h1pool, "h1")
                    with tc.tile_pool(name="qkvw", bufs=1) as qkvw, \
                            tc.tile_pool(name="qkvp", bufs=4,
                                         space="PSUM") as qkvp:
                        wqk_t = [qkvw.tile([128, 1024], f32r, tag=f"wqk{kc}",
                                           name="wqk_t") for kc in range(4)]
                        wv_t = [qkvw.tile([128, 512], f32r, tag=f"wv{kc}",
                                          name="wv_t") for kc in range(4)]
                        for kc in range(4):
                            nc.sync.dma_start(
                                out=wqk_t[kc],
                                in_=wqk[kc * 128:(kc + 1) * 128, :])
                            nc.sync.dma_start(
                                out=wv_t[kc],
                                in_=wv[kc * 128:(kc + 1) * 128, :])
                        for m in range(8):
                            for nh in range(2):
                                s = slice(nh * 512, nh * 512 + 512)
                                pp = qkvp.tile([128, 512], f32, tag="mmqk")
                                for kc in range(4):
                                    mm(pp,
                                       wqk_t[kc][:, m * 128:(m + 1) * 128],
                                       h1[kc][:, s], start=(kc == 0),
                                       stop=(kc == 3))
                                nc.any.tensor_copy(out=qk[m][:, s], in_=pp)
                        for t in range(8):
                            pp = qkvp.tile([128, 512], f32, tag="mmv")
                            for kc in range(4):
                                mm(pp, h1[kc][:, t * 128:(t + 1) * 128],
                                   wv_t[kc], start=(kc == 0), stop=(kc == 3))
                            nc.any.tensor_copy(
                                out=vaug[t][:, :, 0:64],
                                in_=pp[:].rearrange("p (h d) -> p h d", h=8))
                            nc.sync.dma_start(out=vaug[t][:, :, 64:65],
                                              in_=onesb[:].rearrange("p (h o) -> p h o", o=1))
                        if probe:
                            for kc in range(4):
                                nc.sync.dma_start(out=p_h1[kc * 128:(kc + 1) * 128, :], in_=h1[kc].bitcast(f32))
                            for m in range(8):
                                nc.sync.dma_start(out=p_qk[m], in_=qk[m].bitcast(f32))
                            for t2 in range(8):
                                nc.sync.dma_start(out=p_vaug[t2], in_=vaug[t2].bitcast(f32))

                # attention: S^T[j,i], biased softmax over j (partitions)
                with tc.tile_pool(name="attt", bufs=1) as attt, \
                        tc.tile_pool(name="attps", bufs=2,
                                     space="PSUM") as attps, \
                        tc.tile_pool(name="attpo", bufs=1,
                                     space="PSUM") as attpo:
                    masks = [[None] * 3 for _ in range(8)]
                    for jt in range(8):
                        eit = attt.tile([128, V], bf16, tag="eit",
                                        bufs=2, name="eit")
                        nc.sync.dma_start(out=eit,
                                          in_=eiT[jt * 128:(jt + 1) * 128, :])
                        for e in range(3):
                            mk = attt.tile([128, V], bf16,
                                           tag=f"mask{jt}_{e}", bufs=1,
                                           name="mk")
                            nc.vector.tensor_scalar(
                                mk, eit, float(e), None, ALU.is_equal)
                            masks[jt][e] = mk
                    for hg in range(4):
                        ops = [attpo.tile([65, V], f32, tag=f"ops{i}",
                                          bufs=1, name=f"ops{i}")
                               for i in range(2)]
                        for jt in range(8):
                            jsl = slice(jt * 128, jt * 128 + 128)
                            for hi in range(2):
                                h = hg * 2 + hi
                                kt = qk[4 + h // 2][
                                    (h % 2) * 64:(h % 2) * 64 + 64, jsl]
                                S = attps.tile([128, V], f32, tag="mms",
                                               name="S")
                                for nh in range(2):
                                    s = slice(nh * 512, nh * 512 + 512)
                                    qt = qk[h // 2][
                                        (h % 2) * 64:(h % 2) * 64 + 64, s]
                                    nc.tensor.matmul(S[:, s], kt, qt,
                                                     start=True, stop=True)
                                sm1 = attt.tile([128, V], bf16,
                                                tag="sm1", bufs=2,
                                                name="sm1")
                                nc.vector.tensor_scalar(
                                    sm1, masks[jt][1], cb[1][h], None,
                                    ALU.mult)
                                sm2 = attt.tile([128, V], bf16,
                                                tag="sm2", bufs=2,
                                                name="sm2")
                                nc.vector.tensor_scalar(
                                    sm2, masks[jt][2], cb[2][h], None,
                                    ALU.mult)
                                L1 = attt.tile([128, V], bf16,
                                               tag="logits1", bufs=2,
                                               name="L1")
                                nc.vector.scalar_tensor_tensor(
                                    L1, masks[jt][0], cb[0][h], S,
                                    ALU.mult, ALU.add)
                                L2 = attt.tile([128, V], bf16,
                                               tag="logits2", bufs=2,
                                               name="L2")
                                nc.gpsimd.tensor_add(L2, L1, sm1)
                                L3 = attt.tile([128, V], bf16,
                                               tag="logits3", bufs=2,
                                               name="L3")
                                nc.vector.tensor_add(L3, L2, sm2)
                                P = attt.tile([128, V], bf16, tag="probs",
                                              bufs=2, name="P")
                                nc.scalar.activation(P, L3, ACTF.Exp)
                                if probe and jt == 0 and hg == 0 and hi == 0:
                                    nc.sync.dma_start(out=p_L[:],
                                                      in_=L3[:, 0:512])
                                    nc.sync.dma_start(out=p_P[:],
                                                      in_=P[:, 0:512])
                                    nc.sync.dma_start(out=p_m0[:],
                                                      in_=masks[jt][0][:,
                                                                       0:512])
                                for nh in range(2):
                                    s = slice(nh * 512, nh * 512 + 512)
                                    nc.tensor.matmul(
                                        ops[hi][:, s], vaug[jt][:, h, :],
                                        P[:, s], start=(jt == 0),
                                        stop=(jt == 7))
                        # divide by the ones-row sums
                        if probe and hg == 0:
                            opsb = attt.tile([65, 512], f32, tag="opsb",
                                             name="opsb")
                            nc.any.tensor_copy(out=opsb,
                                               in_=ops[0][:, 0:512])
                            nc.sync.dma_start(out=p_ops[:], in_=opsb)
                        for hi in range(2):
                            h = hg * 2 + hi
                            ls = attt.tile([65, V], f32, tag="ls",
                                           bufs=2, name="ls")
                            nc.scalar.copy(ls[64:65, :], ops[hi][64:65, :])
                            lT = attt.tile([128, 8], f32, tag="lT",
                                           bufs=2, name="lT")
                            for c in range(8):
                                nc.sync.dma_start(
                                    out=lT[:, c:c + 1],
                                    in_=ls[64:65,
                                           c * 128:(c + 1) * 128])
                            rlT = attt.tile([128, 8], f32, tag="rlT",
                                            bufs=2, name="rlT")
                            nc.vector.reciprocal(rlT, lT)
                            rl_s = attt.tile([1, V], f32, tag="rls",
                                             bufs=2, name="rl_s")
                            for c in range(8):
                                nc.sync.dma_start(
                                    out=rl_s[0:1, c * 128:(c + 1) * 128],
                                    in_=rlT[:, c:c + 1])
                            rlb = attt.tile([64, V], f32, tag="rlb",
                                            bufs=2, name="rlb")
                            nc.gpsimd.partition_broadcast(rlb, rl_s)
                            nc.vector.tensor_mul(
                                att[h // 2][(h % 2) * 64:(h % 2) * 64 + 64,
                                            :],
                                ops[hi][0:64, :], rlb)

                # proj + residual (in place into xT_t)
                with tc.tile_pool(name="projw", bufs=1) as projw, \
                        tc.tile_pool(name="projp", bufs=4,
                                     space="PSUM") as projp:
                    wp_t = [projw.tile([128, 512], f32r, tag=f"wproj{kc}",
                                       name="wp_t") for kc in range(4)]
                    for kc in range(4):
                        nc.sync.dma_start(
                            out=wp_t[kc],
                            in_=wproj[kc * 128:(kc + 1) * 128, :])
                    for m in range(4):
                        for nh in range(2):
                            s = slice(nh * 512, nh * 512 + 512)
                            pp = projp.tile([128, 512], f32, tag="mmproj")
                            for kc in range(4):
                                mm(pp, wp_t[kc][:, m * 128:(m + 1) * 128],
                                   att[kc][:, s], start=(kc == 0),
                                   stop=(kc == 3))
                            nc.vector.scalar_tensor_tensor(
                                x2[m][:, s], pp, bp_t[:, m:m + 1],
                                xT_t[m][:, s].bitcast(f32), ALU.add,
                                ALU.add)
                    if probe:
                        for kc in range(4):
                            nc.sync.dma_start(out=p_att[kc * 128:(kc + 1) * 128, :], in_=att[kc].bitcast(f32))
                            nc.sync.dma_start(out=p_x2[kc * 128:(kc + 1) * 128, :], in_=x2[kc].bitcast(f32))

            # ---------- MLP branch (xT_t now holds x2) ----------
            with tc.tile_pool(name="mlplife", bufs=1) as mlplife:
                h2 = adaln(x2, 1, mlplife, "h2")
                if probe:
                    for kc in range(4):
                        nc.sync.dma_start(out=p_h2[kc * 128:(kc + 1) * 128, :], in_=h2[kc].bitcast(f32))
                with tc.tile_pool(name="mlpw", bufs=1) as mlpw, \
                        tc.tile_pool(name="mlpt", bufs=1) as mlpt, \
                        tc.tile_pool(name="mlpp", bufs=4,
                                     space="PSUM") as mlpp:
                    wm1_t = [mlpw.tile([128, 2048], f32r, tag=f"wm1{kc}",
                                       name="wm1_t") for kc in range(4)]
                    for kc in range(4):
                        nc.sync.dma_start(
                            out=wm1_t[kc],
                            in_=wm1[kc * 128:(kc + 1) * 128, :])
                    wm2_t = [mlpw.tile([128, 512], f32r, tag=f"wm2{kc}",
                                       name="wm2_t") for kc in range(16)]
                    for kc in range(16):
                        nc.sync.dma_start(
                            out=wm2_t[kc],
                            in_=wm2[kc * 128:(kc + 1) * 128, :])
                    for nh in range(2):
                        s = slice(nh * 512, nh * 512 + 512)
                        g = [mlpt.tile([128, 512], f32r, tag=f"g{m}",
                                       name=f"g{m}") for m in range(16)]
                        for m in range(16):
                            pp = mlpp.tile([128, 512], f32, tag="mmm1")
                            for kc in range(4):
                                mm(pp, wm1_t[kc][:, m * 128:(m + 1) * 128],
                                   h2[kc][:, s], start=(kc == 0),
                                   stop=(kc == 3))
                            nc.scalar.activation(g[m], pp, ACTF.Gelu,
                                                 bias=bm1_t[:, m:m + 1])
                        for m in range(4):
                            pp = mlpp.tile([128, 512], f32, tag="mmm2")
                            for kc in range(16):
                                mm(pp, wm2_t[kc][:, m * 128:(m + 1) * 128],
                                   g[kc], start=(kc == 0), stop=(kc == 15))
                            yt = mlpt.tile([128, 512], f32, tag="yt",
                                           bufs=2, name="yt")
                            nc.vector.scalar_tensor_tensor(
                                yt, pp, bm2_t[:, m:m + 1],
                                x2[m][:, s].bitcast(f32), ALU.add,
                                ALU.add)
                            nc.sync.dma_start(
                                out=yT[m * 128:(m + 1) * 128, s], in_=yt)

    nc.compile()
    return nc


def _make_in_maps(inputs):
    x = np.asarray(inputs["x"], dtype=np.float32)
    cond = np.asarray(inputs["cond"], dtype=np.float32)
    ei = np.asarray(inputs["edge_index"])
    w_qkv = np.asarray(inputs["w_qkv"], dtype=np.float32)

    scale = 1.0 / np.sqrt(HD)
    wqk = w_qkv[:, :2 * D].copy()
    wqk[:, :D] *= scale
    wv = np.ascontiguousarray(w_qkv[:, 2 * D:])
    wada = np.concatenate([inputs["w_ada1"], inputs["w_ada2"]],
                          axis=1).astype(np.float32)
    bada = np.concatenate([inputs["b_ada1"], inputs["b_ada2"]]).astype(
        np.float32).copy()
    bada[:D] += 1.0          # fold the (1 + scale) into ada1 scale bias
    bada[2 * D:3 * D] += 1.0  # and ada2 scale bias

    shared = {
        "onesc": np.ones((128, 8), dtype=np.float32),
        "onesb": np.ones((128, 8), dtype=ml_dtypes.bfloat16),
        "wqk": np.ascontiguousarray(wqk),
        "wv": wv,
        "wada": np.ascontiguousarray(wada),
        "bada": np.ascontiguousarray(bada.reshape(16, 128)),
        "wproj": np.ascontiguousarray(inputs["w_proj"].astype(np.float32)),
        "bproj": np.ascontiguousarray(
            inputs["b_proj"].astype(np.float32).reshape(4, 128)),
        "wm1": np.ascontiguousarray(inputs["w_mlp1"].astype(np.float32)),
        "bm1": np.ascontiguousarray(
            inputs["b_mlp1"].astype(np.float32).reshape(16, 128)),
        "wm2": np.ascontiguousarray(inputs["w_mlp2"].astype(np.float32)),
        "bm2": np.ascontiguousarray(
            inputs["b_mlp2"].astype(np.float32).reshape(4, 128)),
    }
    in_maps = []
    for b in range(B):
        in_maps.append(dict(
            shared,
            xT=np.ascontiguousarray(x[b].T),
            eiT=np.ascontiguousarray(ei[b].T.astype(ml_dtypes.bfloat16)),
            condc=np.ascontiguousarray(cond[b].reshape(4, 128)),
        ))
    return in_maps


def kernel(**inputs):
    from concourse.bass_utils import run_bass_kernel_spmd

    et = np.asarray(inputs["edge_table"], dtype=np.float32)
    cb = [[float(et[e, h] - et[3, h]) for h in range(H)] for e in range(3)]

    key = (et.tobytes(),)
    if key not in _cache:
        _cache[key] = _build_program(cb)
    nc = _cache[key]

    in_maps = _make_in_maps(inputs)
    res = run_bass_kernel_spmd(nc, in_maps, core_ids=list(range(NCORES)))
    out = np.stack([np.ascontiguousarray(res.results[b]["yT"].T)
                    for b in range(B)])
    return out.astype(np.float32)



# revision 6
# speedup vs baseline: 1.7666x; 1.7666x over previous
"""MeshMeanFlowNet block on 8 Trainium2 NeuronCores.

Sharding: data-parallel over B (one batch element per core), no collectives.
Activations are feature-major on device ([feature, token]); the attention
softmax is computed in the transposed layout S^T[j, i] (j = key token on
partitions) with the denominator coming from a ones-row appended to V.

Key structure vs a naive port:
  * AdaLN parameter path (SiLU->Linear on cond) is precomputed on HOST --
    the 4MB wada weight never touches the device.
  * The per-edge-type/per-head softmax bias is applied by ONE custom DVE
    instruction per (head, key-block): P = E * p_h(e) where E = exp(S),
    e = edge type as bf16, and p_h is the cubic polynomial interpolating
    exp(bias[e,h] - bias[0,h]) at e = 0..3 (the exp(-bias[0,h]) factor
    cancels in softmax normalization). No mask tiles, no logit adds.
  * exp runs on the scalar engine over head-pair-wide [128, 2048] PSUM.
  * Score matmuls for the two heads of a pair run concurrently in the PE
    array via row tiling (K=64 each, rows 0-63 / 64-127).
  * All weights travel as bf16; elementwise work is spread over
    vector/gpsimd/scalar engines.
"""

import sys

sys.path.insert(0, "/opt/trn_rl_repo")

import ml_dtypes
import numpy as np

B, V, D, H = 8, 1024, 512, 8
HD = D // H  # 64
NCORES = 8

_cache = {}
_OP = None


def _register_dve_op():
    """Custom DVE op: out = in0 * (1 + in1*(s0 + in1*(s1 + in1*imm2)))."""
    global _OP
    if _OP is not None:
        return _OP
    from concourse import dve_ops
    from concourse.dve_spec import C0, C1, C2, Spec, Src0, Src1, lower
    from concourse.dve_uop import DveOpSpec

    name = "EDGE_SOFTMAX_SCALE_ANT"
    for o in dve_ops.OPS:
        if o.name == name:
            _OP = o
            return o

    body = Src0 + Src0 * (Src1 * (C0 + Src1 * (C1 + Src1 * C2)))
    spec = Spec(
        body=body,
        reference=lambda in0, in1, s0, s1, imm2: in0
        + in0 * (in1 * (s0 + in1 * (s1 + in1 * imm2))),
    )
    opcode = dve_ops._CUSTOM_DVE_ROW_BASE + len(dve_ops.OPS)
    shas = {}
    for ver in ("v3",):
        compiled = DveOpSpec(
            name=name, opcode=opcode, uops=lower(spec, ver=ver), rd1_en=True
        )
        shas[ver] = compiled.sha(ver)
    op = dve_ops.DveOp(name, spec, subdim=False, uops_sha=shas)
    dve_ops.OPS.append(op)
    dve_ops._SUB_OPCODE_FOR_NAME[name] = opcode
    dve_ops.CUSTOM_DVE_SPECS[name] = spec
    _OP = op
    return op


def _build_program(qc):
    """qc: [H][3] cubic coefficients for the per-head edge-bias polynomial."""
    import contextlib

    import concourse.bacc as bacc
    import concourse.tile as tile
    from concourse import mybir

    op = _register_dve_op()

    f32 = mybir.dt.float32
    f32r = mybir.dt.float32r
    bf16 = mybir.dt.bfloat16
    ALU = mybir.AluOpType
    ACTF = mybir.ActivationFunctionType

    nc = bacc.Bacc("TRN2", target_bir_lowering=False, debug=False,
                   num_devices=NCORES)

    # ---- DRAM I/O (per-core shard, host pre-laid-out) ----
    xT = nc.dram_tensor("xT", [D, V], f32r, kind="ExternalInput")
    eiT = nc.dram_tensor("eiT", [V, V], bf16, kind="ExternalInput")  # [j, i]
    paramsd = nc.dram_tensor("paramsd", [128, 16], f32, kind="ExternalInput")
    wqk = nc.dram_tensor("wqk", [D, 1024], bf16, kind="ExternalInput")
    wv = nc.dram_tensor("wv", [D, 512], bf16, kind="ExternalInput")
    wproj = nc.dram_tensor("wproj", [D, D], bf16, kind="ExternalInput")
    wm1 = nc.dram_tensor("wm1", [D, 2048], bf16, kind="ExternalInput")
    wm2 = nc.dram_tensor("wm2", [2048, D], bf16, kind="ExternalInput")
    bprojd = nc.dram_tensor("bprojd", [128, 4], f32, kind="ExternalInput")
    bm1d = nc.dram_tensor("bm1d", [128, 16], f32, kind="ExternalInput")
    bm2d = nc.dram_tensor("bm2d", [128, 4], f32, kind="ExternalInput")
    onesf = nc.dram_tensor("onesf", [128, 1], f32r, kind="ExternalInput")
    onesb = nc.dram_tensor("onesb", [128, 8], bf16, kind="ExternalInput")
    yT = nc.dram_tensor("yT", [D, V], f32, kind="ExternalOutput")

    def mm(out, lhsT, rhs, **kw):
        nc.tensor.matmul(out, lhsT.bitcast(f32r), rhs.bitcast(f32r), **kw)

    def mmb(out, lhsT, rhs, **kw):
        nc.tensor.matmul(out, lhsT, rhs, **kw)

    with tile.TileContext(nc) as tc:
        with contextlib.ExitStack() as ctx:
            persist = ctx.enter_context(tc.tile_pool(name="persist", bufs=1))

            ones = persist.tile([128, 1], f32r, tag="ones")
            nc.sync.dma_start(out=ones, in_=onesf[:, :])
            epst = persist.tile([1, 1], f32, tag="eps")
            nc.vector.memset(epst, 1e-5)

            params = persist.tile([128, 16], f32, tag="params")
            nc.sync.dma_start(out=params, in_=paramsd[:, :])
            bp_t = persist.tile([128, 4], f32, tag="bproj")
            nc.sync.dma_start(out=bp_t, in_=bprojd[:, :])
            bm1_t = persist.tile([128, 16], f32, tag="bm1")
            nc.sync.dma_start(out=bm1_t, in_=bm1d[:, :])
            bm2_t = persist.tile([128, 4], f32, tag="bm2")
            nc.sync.dma_start(out=bm2_t, in_=bm2d[:, :])

            # x (feature-major, f32 for the residual adds)
            xT_t = [persist.tile([128, V], f32r, tag=f"xT{kc}",
                                 name=f"xT_t{kc}") for kc in range(4)]
            for kc in range(4):
                nc.sync.dma_start(out=xT_t[kc],
                                  in_=xT[kc * 128:(kc + 1) * 128, :])

            # all weights up-front (DMA overlaps LN1/qkv compute)
            wqk_t = [persist.tile([128, 1024], bf16, tag=f"wqk{kc}",
                                  name="wqk_t") for kc in range(4)]
            wv_t = [persist.tile([128, 512], bf16, tag=f"wv{kc}",
                                 name="wv_t") for kc in range(4)]
            wp_t = [persist.tile([128, 512], bf16, tag=f"wproj{kc}",
                                 name="wp_t") for kc in range(4)]
            wm1_t = [persist.tile([128, 2048], bf16, tag=f"wm1{kc}",
                                  name="wm1_t") for kc in range(4)]
            wm2_t = [persist.tile([128, 512], bf16, tag=f"wm2{kc}",
                                  name="wm2_t") for kc in range(16)]
            for kc in range(4):
                nc.sync.dma_start(out=wqk_t[kc],
                                  in_=wqk[kc * 128:(kc + 1) * 128, :])
                nc.sync.dma_start(out=wv_t[kc],
                                  in_=wv[kc * 128:(kc + 1) * 128, :])
                nc.sync.dma_start(out=wp_t[kc],
                                  in_=wproj[kc * 128:(kc + 1) * 128, :])
                nc.sync.dma_start(out=wm1_t[kc],
                                  in_=wm1[kc * 128:(kc + 1) * 128, :])
            for kc in range(16):
                nc.sync.dma_start(out=wm2_t[kc],
                                  in_=wm2[kc * 128:(kc + 1) * 128, :])

            # edge types (bf16 exact for values 0..3)
            eit = [persist.tile([128, V], bf16, tag=f"eit{jt}",
                                name=f"eit{jt}") for jt in range(8)]
            for jt in range(8):
                nc.sync.dma_start(out=eit[jt],
                                  in_=eiT[jt * 128:(jt + 1) * 128, :])

            x2 = [persist.tile([128, V], f32r, tag=f"x2_{kc}",
                               name=f"x2_{kc}") for kc in range(4)]
            att = [persist.tile([128, V], bf16, tag=f"att{g}",
                                name=f"att{g}") for g in range(4)]

            vec_or_gp = [nc.vector, nc.gpsimd]

            def adaln(src_tiles, ln_idx, dst_pool, out_tag, warm_after):
                """LayerNorm over features (partitions) + adaptive affine.
                src f32 tiles -> 4 bf16 tiles. warm_after: psum tile to run
                PE warm-keeper matmuls into during the scalar interlude."""
                out = [dst_pool.tile([128, V], bf16, tag=f"{out_tag}{kc}",
                                     name=f"ln_{out_tag}{kc}")
                       for kc in range(4)]
                with tc.tile_pool(name="lnt", bufs=1) as lnt, \
                        tc.tile_pool(name="lnp", bufs=1, space="PSUM") as lnp:
                    ps_s = lnp.tile([1, V], f32, tag="lnsum")
                    ps_q = lnp.tile([1, V], f32, tag="lnsqsum")
                    for kc in range(4):
                        sq = lnt.tile([128, V], f32r, tag="lnsq", bufs=2,
                                      name="sq")
                        nc.gpsimd.tensor_mul(
                            sq, src_tiles[kc].bitcast(f32),
                            src_tiles[kc].bitcast(f32))
                        for nh in range(2):
                            s = slice(nh * 512, nh * 512 + 512)
                            mm(ps_s[:, s], ones, src_tiles[kc][:, s],
                               start=(kc == 0), stop=(kc == 3))
                            mm(ps_q[:, s], ones, sq[:, s],
                               start=(kc == 0), stop=(kc == 3))
                    # PE warm-keepers: ~20 chained matmuls (~4.3us) bridging
                    # the scalar interlude so HAM stays at full clock.
                    wk = lnp.tile([1, 512], f32, tag="warmkeep")
                    for w in range(20):
                        mm(wk, ones, warm_after[:, 0:512],
                           start=(w == 0), stop=(w == 19))
                    mean = lnt.tile([1, V], f32, tag="mean")
                    nc.scalar.mul(mean, ps_s, 1.0 / D)
                    msq = lnt.tile([1, V], f32, tag="msq")
                    nc.vector.tensor_mul(msq, mean, mean)
                    std = lnt.tile([1, V], f32, tag="std")
                    nc.vector.scalar_tensor_tensor(std, ps_q, 1.0 / D, msq,
                                                   ALU.mult, ALU.subtract)
                    nc.scalar.activation(std, std, ACTF.Sqrt, bias=epst)
                    r = lnt.tile([1, V], f32, tag="r")
                    nc.vector.reciprocal(r, std)
                    mr = lnt.tile([1, V], f32, tag="mr")
                    nc.gpsimd.tensor_mul(mr, mean, r)
                    rb = lnt.tile([128, V], f32, tag="rb")
                    nc.gpsimd.partition_broadcast(rb, r)
                    mrb = lnt.tile([128, V], f32, tag="mrb")
                    nc.gpsimd.partition_broadcast(mrb, mr)
                    for kc in range(4):
                        smd = ln_idx * 8 + kc
                        tmd = ln_idx * 8 + 4 + kc
                        u = lnt.tile([128, V], f32, tag="lnu", bufs=2,
                                     name="u")
                        nc.gpsimd.tensor_mul(u, src_tiles[kc].bitcast(f32), rb)
                        u2 = lnt.tile([128, V], f32, tag="lnu2", bufs=2,
                                      name="u2")
                        nc.vector.scalar_tensor_tensor(
                            u2, mrb, -1.0, u, ALU.mult, ALU.add)
                        if kc % 2 == 0:
                            nc.scalar.activation(
                                out[kc], u2, ACTF.Identity,
                                bias=params[:, tmd:tmd + 1],
                                scale=params[:, smd:smd + 1])
                        else:
                            nc.vector.tensor_scalar(
                                out[kc], u2, params[:, smd:smd + 1],
                                params[:, tmd:tmd + 1], ALU.mult, ALU.add)
                return out

            # ---- attention lifetime ----
            with tc.tile_pool(name="attlife", bufs=1) as attlife:
                qk = [attlife.tile([128, V], bf16, tag=f"qk{m}",
                                   name=f"qk{m}") for m in range(8)]
                vaug = [attlife.tile([128, 8, 65], bf16, tag=f"vaug{t}",
                                     name=f"vaug{t}") for t in range(8)]

                # h1 = AdaLN1(x); qk feature-major; v token-major
                with tc.tile_pool(name="h1pool", bufs=1) as h1pool:
                    h1 = adaln(xT_t, 0, h1pool, "h1", xT_t[0])
                    with tc.tile_pool(name="qkvp", bufs=4,
                                      space="PSUM") as qkvp:
                        cp_rot = 0
                        for m in range(8):
                            for nh in range(2):
                                s = slice(nh * 512, nh * 512 + 512)
                                pp = qkvp.tile([128, 512], f32, tag="mmqk")
                                for kc in range(4):
                                    mmb(pp,
                                        wqk_t[kc][:, m * 128:(m + 1) * 128],
                                        h1[kc][:, s], start=(kc == 0),
                                        stop=(kc == 3))
                                if cp_rot % 2 == 1:
                                    nc.scalar.copy(qk[m][:, s], pp)
                                else:
                                    nc.vector.tensor_copy(
                                        out=qk[m][:, s], in_=pp)
                                cp_rot += 1
                        for t in range(8):
                            pp = qkvp.tile([128, 512], f32, tag="mmv")
                            for kc in range(4):
                                mmb(pp, h1[kc][:, t * 128:(t + 1) * 128],
                                    wv_t[kc], start=(kc == 0),
                                    stop=(kc == 3))
                            nc.vector.tensor_copy(
                                out=vaug[t][:, :, 0:64],
                                in_=pp[:].rearrange("p (h d) -> p h d", h=8))
                            nc.sync.dma_start(
                                out=vaug[t][:, :, 64:65],
                                in_=onesb[:].rearrange("p (h o) -> p h o",
                                                       o=1))

                # attention core
                with tc.tile_pool(name="attt", bufs=1) as attt, \
                        tc.tile_pool(name="attps", bufs=1,
                                     space="PSUM") as attps:
                    S2 = attps.tile([128, 2048], f32, tag="S2")
                    ops = [attps.tile([65, V], f32, tag=f"ops{i}",
                                      name=f"ops{i}") for i in range(2)]
                    for hg in range(4):
                        kt = qk[4 + hg]
                        qt = qk[hg]
                        for jt in range(8):
                            jsl = slice(jt * 128, jt * 128 + 128)
                            for nh in range(2):
                                s = slice(nh * 512, nh * 512 + 512)
                                mmb(S2[:, nh * 512:nh * 512 + 512],
                                    kt[0:64, jsl], qt[0:64, s],
                                    start=True, stop=True,
                                    tile_position=(0, 0))
                                mmb(S2[:, 1024 + nh * 512:
                                       1024 + nh * 512 + 512],
                                    kt[64:128, jsl], qt[64:128, s],
                                    start=True, stop=True,
                                    tile_position=(64, 0))
                            E2 = attt.tile([128, 2048], bf16, tag="E2",
                                           bufs=2, name="E2")
                            nc.scalar.activation(E2, S2, ACTF.Exp)
                            for hi in range(2):
                                h = hg * 2 + hi
                                P = attt.tile([128, V], bf16, tag="P",
                                              bufs=3, name="P")
                                nc.vector._custom_dve(
                                    op, out=P,
                                    in0=E2[:, hi * 1024:hi * 1024 + 1024],
                                    in1=eit[jt],
                                    s0=qc[h][0], s1=qc[h][1],
                                    imm2=qc[h][2])
                                for nh in range(2):
                                    s = slice(nh * 512, nh * 512 + 512)
                                    mmb(ops[hi][:, s], vaug[jt][:, h, :],
                                        P[:, s], start=(jt == 0),
                                        stop=(jt == 7))
                        # normalize the head pair
                        rl2 = [attt.tile([1, V], f32, tag=f"rl2_{hi}",
                                         bufs=2, name="rl2")
                               for hi in range(2)]
                        for hi in range(2):
                            nc.vector.reciprocal(rl2[hi],
                                                 ops[hi][64:65, :])
                        for hi in range(2):
                            rlb = attt.tile([64, V], f32, tag=f"rlb{hi}",
                                            bufs=2, name="rlb")
                            nc.gpsimd.partition_broadcast(
                                rlb, rl2[hi])
                            nc.vector.tensor_mul(
                                att[hg][hi * 64:hi * 64 + 64, :],
                                ops[hi][0:64, :], rlb)

                # proj + residual -> x2
                with tc.tile_pool(name="projp", bufs=4,
                                  space="PSUM") as projp:
                    for m in range(4):
                        for nh in range(2):
                            s = slice(nh * 512, nh * 512 + 512)
                            pp = projp.tile([128, 512], f32, tag="mmproj")
                            for kc in range(4):
                                mmb(pp, wp_t[kc][:, m * 128:(m + 1) * 128],
                                    att[kc][:, s], start=(kc == 0),
                                    stop=(kc == 3))
                            nc.vector.scalar_tensor_tensor(
                                x2[m][:, s], pp, bp_t[:, m:m + 1],
                                xT_t[m][:, s].bitcast(f32), ALU.add,
                                ALU.add)

            # ---- MLP branch ----
            with tc.tile_pool(name="mlplife", bufs=1) as mlplife:
                h2 = adaln(x2, 1, mlplife, "h2", x2[0])
                with tc.tile_pool(name="mlpt", bufs=1) as mlpt, \
                        tc.tile_pool(name="mlpp", bufs=4,
                                     space="PSUM") as mlpp:
                    for nh in range(2):
                        s = slice(nh * 512, nh * 512 + 512)
                        g = [mlpt.tile([128, 512], bf16, tag=f"g{m}",
                                       name=f"g{m}") for m in range(16)]
                        for m in range(16):
                            pp = mlpp.tile([128, 512], f32, tag="mmm1")
                            for kc in range(4):
                                mmb(pp, wm1_t[kc][:, m * 128:(m + 1) * 128],
                                    h2[kc][:, s], start=(kc == 0),
                                    stop=(kc == 3))
                            nc.scalar.activation(g[m], pp, ACTF.Gelu,
                                                 bias=bm1_t[:, m:m + 1])
                        for m in range(4):
                            pp = mlpp.tile([128, 512], f32, tag="mmm2")
                            for kc in range(16):
                                mmb(pp, wm2_t[kc][:, m * 128:(m + 1) * 128],
                                    g[kc], start=(kc == 0), stop=(kc == 15))
                            yt = mlpt.tile([128, 512], f32, tag="yt",
                                           bufs=2, name="yt")
                            nc.vector.scalar_tensor_tensor(
                                yt, pp, bm2_t[:, m:m + 1],
                                x2[m][:, s].bitcast(f32), ALU.add, ALU.add)
                            nc.sync.dma_start(
                                out=yT[m * 128:(m + 1) * 128, s], in_=yt)

    nc.compile()
    return nc


def _poly_coeffs(edge_table):
    """Per-head cubic p(e) = 1 + e*(q0 + e*(q1 + e*q2)) with
    p(e) = exp(table[e,h] - table[0,h]) for e = 0..3."""
    et = np.asarray(edge_table, dtype=np.float64)
    A = np.array([[1.0, 1.0, 1.0],
                  [1.0, 2.0, 4.0],
                  [1.0, 3.0, 9.0]])
    qc = []
    for h in range(H):
        g = np.exp(et[:, h] - et[0, h])
        rhs = np.array([(g[1] - 1.0) / 1.0,
                        (g[2] - 1.0) / 2.0,
                        (g[3] - 1.0) / 3.0])
        q = np.linalg.solve(A, rhs)
        # verify interpolation
        e = np.arange(4.0)
        p = 1.0 + e * (q[0] + e * (q[1] + e * q[2]))
        assert np.abs(p - g).max() < 1e-9
        qc.append([float(v) for v in q])
    return qc


def _silu(v):
    return v / (1.0 + np.exp(-v))


def _make_in_maps(inputs):
    x = np.asarray(inputs["x"], dtype=np.float32)
    cond = np.asarray(inputs["cond"], dtype=np.float32)
    ei = np.asarray(inputs["edge_index"])
    w_qkv = np.asarray(inputs["w_qkv"], dtype=np.float32)

    scale = 1.0 / np.sqrt(HD)
    wqk = w_qkv[:, :2 * D].copy()
    wqk[:, :D] *= scale
    wv = np.ascontiguousarray(w_qkv[:, 2 * D:])

    # host-side AdaLN parameter path: params[b] = silu(cond[b]) @ w_ada + b
    sc = _silu(cond)  # [B, Dc]
    p1 = sc @ np.asarray(inputs["w_ada1"], dtype=np.float32) \
        + np.asarray(inputs["b_ada1"], dtype=np.float32)  # [B, 2D]
    p2 = sc @ np.asarray(inputs["w_ada2"], dtype=np.float32) \
        + np.asarray(inputs["b_ada2"], dtype=np.float32)
    p1[:, :D] += 1.0  # fold the (1 + scale)
    p2[:, :D] += 1.0
    # params[b] layout [128, 16]: col ln*8 + kind*4 + kc, feature kc*128+p
    paramsb = np.empty((B, 128, 16), dtype=np.float32)
    for ln, p in ((0, p1), (1, p2)):
        s_part = p[:, :D].reshape(B, 4, 128)   # [B, kc, p]
        t_part = p[:, D:].reshape(B, 4, 128)
        paramsb[:, :, ln * 8 + 0:ln * 8 + 4] = s_part.transpose(0, 2, 1)
        paramsb[:, :, ln * 8 + 4:ln * 8 + 8] = t_part.transpose(0, 2, 1)

    shared = {
        "onesf": np.ones((128, 1), dtype=np.float32),
        "onesb": np.ones((128, 8), dtype=ml_dtypes.bfloat16),
        "wqk": wqk.astype(ml_dtypes.bfloat16),
        "wv": wv.astype(ml_dtypes.bfloat16),
        "wproj": np.asarray(inputs["w_proj"],
                            dtype=np.float32).astype(ml_dtypes.bfloat16),
        "wm1": np.asarray(inputs["w_mlp1"],
                          dtype=np.float32).astype(ml_dtypes.bfloat16),
        "wm2": np.asarray(inputs["w_mlp2"],
                          dtype=np.float32).astype(ml_dtypes.bfloat16),
        "bprojd": np.ascontiguousarray(
            np.asarray(inputs["b_proj"],
                       dtype=np.float32).reshape(4, 128).T),
        "bm1d": np.ascontiguousarray(
            np.asarray(inputs["b_mlp1"],
                       dtype=np.float32).reshape(16, 128).T),
        "bm2d": np.ascontiguousarray(
            np.asarray(inputs["b_mlp2"],
                       dtype=np.float32).reshape(4, 128).T),
    }
    in_maps = []
    for b in range(B):
        in_maps.append(dict(
            shared,
            xT=np.ascontiguousarray(x[b].T),
            eiT=np.ascontiguousarray(ei[b].T.astype(ml_dtypes.bfloat16)),
            paramsd=np.ascontiguousarray(paramsb[b]),
        ))
    return in_maps


def kernel(**inputs):
    from concourse.bass_utils import run_bass_kernel_spmd

    et = np.asarray(inputs["edge_table"], dtype=np.float32)
    qc = _poly_coeffs(et)

    key = (et.tobytes(),)
    if key not in _cache:
        _cache[key] = _build_program(qc)
    nc = _cache[key]

    in_maps = _make_in_maps(inputs)
    res = run_bass_kernel_spmd(nc, in_maps, core_ids=list(range(NCORES)))
    out = np.stack([np.ascontiguousarray(res.results[b]["yT"].T)
                    for b in range(B)])
    return out.astype(np.float32)


# revision 8
# speedup vs baseline: 1.7810x; 1.0081x over previous
"""MeshMeanFlowNet block on 8 Trainium2 NeuronCores.

Sharding: data-parallel over B (one batch element per core), no collectives.
Activations are feature-major on device ([feature, token]); the attention
softmax is computed in the transposed layout S^T[j, i] (j = key token on
partitions) with the denominator coming from a ones-row appended to V.

Key structure vs a naive port:
  * AdaLN parameter path (SiLU->Linear on cond) is precomputed on HOST --
    the 4MB wada weight never touches the device.
  * The per-edge-type/per-head softmax bias is applied by ONE custom DVE
    instruction per (head, key-block): P = E * p_h(e) where E = exp(S),
    e = edge type as bf16, and p_h is the cubic polynomial interpolating
    exp(bias[e,h] - bias[0,h]) at e = 0..3 (the exp(-bias[0,h]) factor
    cancels in softmax normalization). No mask tiles, no logit adds.
  * exp runs on the scalar engine over head-pair-wide [128, 2048] PSUM.
  * Score matmuls for the two heads of a pair run concurrently in the PE
    array via row tiling (K=64 each, rows 0-63 / 64-127).
  * All weights travel as bf16; elementwise work is spread over
    vector/gpsimd/scalar engines.
"""

import sys

sys.path.insert(0, "/opt/trn_rl_repo")

import ml_dtypes
import numpy as np

B, V, D, H = 8, 1024, 512, 8
HD = D // H  # 64
NCORES = 8

_cache = {}
_OP = None


def _register_dve_op():
    """Custom DVE op: out = in0 * (1 + in1*(s0 + in1*(s1 + in1*imm2)))."""
    global _OP
    if _OP is not None:
        return _OP
    from concourse import dve_ops
    from concourse.dve_spec import C0, C1, C2, Spec, Src0, Src1, lower
    from concourse.dve_uop import DveOpSpec

    name = "EDGE_SOFTMAX_SCALE_ANT"
    for o in dve_ops.OPS:
        if o.name == name:
            _OP = o
            return o

    body = Src0 + Src0 * (Src1 * (C0 + Src1 * (C1 + Src1 * C2)))
    spec = Spec(
        body=body,
        reference=lambda in0, in1, s0, s1, imm2: in0
        + in0 * (in1 * (s0 + in1 * (s1 + in1 * imm2))),
    )
    opcode = dve_ops._CUSTOM_DVE_ROW_BASE + len(dve_ops.OPS)
    shas = {}
    for ver in ("v3",):
        compiled = DveOpSpec(
            name=name, opcode=opcode, uops=lower(spec, ver=ver), rd1_en=True
        )
        shas[ver] = compiled.sha(ver)
    op = dve_ops.DveOp(name, spec, subdim=False, uops_sha=shas)
    dve_ops.OPS.append(op)
    dve_ops._SUB_OPCODE_FOR_NAME[name] = opcode
    dve_ops.CUSTOM_DVE_SPECS[name] = spec
    _OP = op
    return op


def _build_program(qc):
    """qc: [H][3] cubic coefficients for the per-head edge-bias polynomial."""
    import contextlib

    import concourse.bacc as bacc
    import concourse.tile as tile
    from concourse import mybir

    op = _register_dve_op()

    f32 = mybir.dt.float32
    f32r = mybir.dt.float32r
    bf16 = mybir.dt.bfloat16
    ALU = mybir.AluOpType
    ACTF = mybir.ActivationFunctionType

    nc = bacc.Bacc("TRN2", target_bir_lowering=False, debug=False,
                   num_devices=NCORES)

    # ---- DRAM I/O (per-core shard, host pre-laid-out) ----
    xT = nc.dram_tensor("xT", [D, V], f32r, kind="ExternalInput")
    eiT = nc.dram_tensor("eiT", [V, V], bf16, kind="ExternalInput")  # [j, i]
    wqk = nc.dram_tensor("wqk", [D, 1024], bf16, kind="ExternalInput")
    wv = nc.dram_tensor("wv", [D, 512], bf16, kind="ExternalInput")
    wproj = nc.dram_tensor("wproj", [D, D], bf16, kind="ExternalInput")
    wm1 = nc.dram_tensor("wm1", [D, 2048], bf16, kind="ExternalInput")
    wm2 = nc.dram_tensor("wm2", [2048, D], bf16, kind="ExternalInput")
    bprojd = nc.dram_tensor("bprojd", [128, 4], f32, kind="ExternalInput")
    bm1d = nc.dram_tensor("bm1d", [128, 16], f32, kind="ExternalInput")
    bm2d = nc.dram_tensor("bm2d", [128, 4], f32, kind="ExternalInput")
    cqkd = nc.dram_tensor("cqkd", [128, 8], f32, kind="ExternalInput")
    cvbd = nc.dram_tensor("cvbd", [128, 520], bf16, kind="ExternalInput")
    onesf = nc.dram_tensor("onesf", [128, 1], f32r, kind="ExternalInput")
    onesb = nc.dram_tensor("onesb", [128, 8], bf16, kind="ExternalInput")
    yT = nc.dram_tensor("yT", [D, V], f32, kind="ExternalOutput")

    def mm(out, lhsT, rhs, **kw):
        nc.tensor.matmul(out, lhsT.bitcast(f32r), rhs.bitcast(f32r), **kw)

    def mmb(out, lhsT, rhs, **kw):
        nc.tensor.matmul(out, lhsT, rhs, **kw)

    with tile.TileContext(nc) as tc:
        with contextlib.ExitStack() as ctx:
            persist = ctx.enter_context(tc.tile_pool(name="persist", bufs=1))

            ones = persist.tile([128, 1], f32r, tag="ones")
            nc.sync.dma_start(out=ones, in_=onesf[:, :])
            epst = persist.tile([1, 1], f32, tag="eps")
            nc.vector.memset(epst, 1e-5)

            cqk = persist.tile([128, 8], f32, tag="cqk")
            nc.sync.dma_start(out=cqk, in_=cqkd[:, :])
            cvb = persist.tile([128, 8, 65], bf16, tag="cvb")
            nc.sync.dma_start(out=cvb,
                              in_=cvbd[:].rearrange("p (h c) -> p h c", h=8))
            bp_t = persist.tile([128, 4], f32, tag="bproj")
            nc.sync.dma_start(out=bp_t, in_=bprojd[:, :])
            bm1_t = persist.tile([128, 16], f32, tag="bm1")
            nc.sync.dma_start(out=bm1_t, in_=bm1d[:, :])
            bm2_t = persist.tile([128, 4], f32, tag="bm2")
            nc.sync.dma_start(out=bm2_t, in_=bm2d[:, :])

            # x (feature-major, f32 for the residual adds)
            xT_t = [persist.tile([128, V], f32r, tag=f"xT{kc}",
                                 name=f"xT_t{kc}") for kc in range(4)]
            for kc in range(4):
                nc.sync.dma_start(out=xT_t[kc],
                                  in_=xT[kc * 128:(kc + 1) * 128, :])

            # all weights up-front (DMA overlaps LN1/qkv compute)
            wqk_t = [persist.tile([128, 1024], bf16, tag=f"wqk{kc}",
                                  name="wqk_t") for kc in range(4)]
            wv_t = [persist.tile([128, 512], bf16, tag=f"wv{kc}",
                                 name="wv_t") for kc in range(4)]
            wp_t = [persist.tile([128, 512], bf16, tag=f"wproj{kc}",
                                 name="wp_t") for kc in range(4)]
            wm1_t = [persist.tile([128, 2048], bf16, tag=f"wm1{kc}",
                                  name="wm1_t") for kc in range(4)]
            wm2_t = [persist.tile([128, 512], bf16, tag=f"wm2{kc}",
                                  name="wm2_t") for kc in range(16)]
            for kc in range(4):
                nc.sync.dma_start(out=wqk_t[kc],
                                  in_=wqk[kc * 128:(kc + 1) * 128, :])
                nc.sync.dma_start(out=wv_t[kc],
                                  in_=wv[kc * 128:(kc + 1) * 128, :])
                nc.sync.dma_start(out=wp_t[kc],
                                  in_=wproj[kc * 128:(kc + 1) * 128, :])
                nc.sync.dma_start(out=wm1_t[kc],
                                  in_=wm1[kc * 128:(kc + 1) * 128, :])
            for kc in range(16):
                nc.sync.dma_start(out=wm2_t[kc],
                                  in_=wm2[kc * 128:(kc + 1) * 128, :])

            # edge types (bf16 exact for values 0..3)
            eit = [persist.tile([128, V], bf16, tag=f"eit{jt}",
                                name=f"eit{jt}") for jt in range(8)]
            for jt in range(8):
                nc.sync.dma_start(out=eit[jt],
                                  in_=eiT[jt * 128:(jt + 1) * 128, :])

            x2 = [persist.tile([128, V], f32r, tag=f"x2_{kc}",
                               name=f"x2_{kc}") for kc in range(4)]
            att = [persist.tile([128, V], bf16, tag=f"att{g}",
                                name=f"att{g}") for g in range(4)]

            vec_or_gp = [nc.vector, nc.gpsimd]

            def adaln(src_tiles, dst_pool, out_tag, warm_after):
                """LayerNorm over features (partitions). The adaptive affine
                (scale/shift from cond) is folded into the host-scaled
                weights of the consumer matmuls, so the output here is just
                (x - mean) / std, in bf16."""
                out = [dst_pool.tile([128, V], bf16, tag=f"{out_tag}{kc}",
                                     name=f"ln_{out_tag}{kc}")
                       for kc in range(4)]
                with tc.tile_pool(name="lnt", bufs=1) as lnt, \
                        tc.tile_pool(name="lnp", bufs=1, space="PSUM") as lnp:
                    ps_s = lnp.tile([1, V], f32, tag="lnsum")
                    ps_q = lnp.tile([1, V], f32, tag="lnsqsum")
                    for kc in range(4):
                        sq = lnt.tile([128, V], f32r, tag="lnsq", bufs=2,
                                      name="sq")
                        nc.gpsimd.tensor_mul(
                            sq, src_tiles[kc].bitcast(f32),
                            src_tiles[kc].bitcast(f32))
                        for nh in range(2):
                            s = slice(nh * 512, nh * 512 + 512)
                            mm(ps_s[:, s], ones, src_tiles[kc][:, s],
                               start=(kc == 0), stop=(kc == 3))
                            mm(ps_q[:, s], ones, sq[:, s],
                               start=(kc == 0), stop=(kc == 3))
                    # PE warm-keepers bridging the scalar interlude
                    wk = lnp.tile([1, 512], f32, tag="warmkeep")
                    for w in range(28):
                        mm(wk, ones, warm_after[:, 0:512],
                           start=(w == 0), stop=(w == 27))
                    mean = lnt.tile([1, V], f32, tag="mean")
                    nc.scalar.mul(mean, ps_s, 1.0 / D)
                    msq = lnt.tile([1, V], f32, tag="msq")
                    nc.vector.tensor_mul(msq, mean, mean)
                    var = lnt.tile([1, V], f32, tag="var")
                    nc.vector.scalar_tensor_tensor(var, ps_q, 1.0 / D, msq,
                                                   ALU.mult, ALU.subtract)
                    r = lnt.tile([1, V], f32, tag="r")
                    nc.scalar.activation(r, var, ACTF.Abs_reciprocal_sqrt,
                                         bias=epst)
                    mr = lnt.tile([1, V], f32, tag="mr")
                    nc.gpsimd.tensor_mul(mr, mean, r)
                    rb = lnt.tile([128, V], f32, tag="rb")
                    nc.gpsimd.partition_broadcast(rb, r)
                    mrb = lnt.tile([128, V], f32, tag="mrb")
                    nc.gpsimd.partition_broadcast(mrb, mr)
                    for kc in range(4):
                        u = lnt.tile([128, V], f32, tag="lnu", bufs=2,
                                     name="u")
                        nc.gpsimd.tensor_mul(u, src_tiles[kc].bitcast(f32),
                                             rb)
                        nc.vector.scalar_tensor_tensor(
                            out[kc], mrb, -1.0, u, ALU.mult, ALU.add)
                return out

            # ---- attention lifetime ----
            with tc.tile_pool(name="attlife", bufs=1) as attlife:
                qk = [attlife.tile([128, V], bf16, tag=f"qk{m}",
                                   name=f"qk{m}") for m in range(8)]
                vaug = [attlife.tile([128, 8, 65], bf16, tag=f"vaug{t}",
                                     name=f"vaug{t}") for t in range(8)]

                # h1 = AdaLN1(x); qk feature-major; v token-major
                with tc.tile_pool(name="h1pool", bufs=1) as h1pool:
                    h1 = adaln(xT_t, h1pool, "h1", xT_t[0])
                    with tc.tile_pool(name="qkvp", bufs=4,
                                      space="PSUM") as qkvp:
                        cp_rot = 0
                        for m in range(8):
                            for nh in range(2):
                                s = slice(nh * 512, nh * 512 + 512)
                                pp = qkvp.tile([128, 512], f32, tag="mmqk")
                                for kc in range(4):
                                    mmb(pp,
                                        wqk_t[kc][:, m * 128:(m + 1) * 128],
                                        h1[kc][:, s], start=(kc == 0),
                                        stop=(kc == 3))
                                if cp_rot % 2 == 1:
                                    nc.scalar.activation(
                                        qk[m][:, s], pp, ACTF.Identity,
                                        bias=cqk[:, m:m + 1])
                                else:
                                    nc.vector.tensor_scalar(
                                        qk[m][:, s], pp, 1.0,
                                        cqk[:, m:m + 1], ALU.mult, ALU.add)
                                cp_rot += 1
                        for t in range(8):
                            pp = qkvp.tile([128, 512], f32, tag="mmv")
                            for kc in range(4):
                                mmb(pp, h1[kc][:, t * 128:(t + 1) * 128],
                                    wv_t[kc], start=(kc == 0),
                                    stop=(kc == 3))
                            nc.vector.tensor_add(
                                vaug[t][:, :, 0:64],
                                pp[:].rearrange("p (h d) -> p h d", h=8),
                                cvb[:, :, 0:64])
                            nc.sync.dma_start(
                                out=vaug[t][:, :, 64:65],
                                in_=onesb[:].rearrange("p (h o) -> p h o",
                                                       o=1))

                # attention core
                with tc.tile_pool(name="attt", bufs=1) as attt, \
                        tc.tile_pool(name="attps", bufs=1,
                                     space="PSUM") as attps:
                    S2 = attps.tile([128, 2048], f32, tag="S2")
                    ops = [attps.tile([65, V], f32, tag=f"ops{i}",
                                      name=f"ops{i}") for i in range(2)]
                    for hg in range(4):
                        kt = qk[4 + hg]
                        qt = qk[hg]
                        for jt in range(8):
                            jsl = slice(jt * 128, jt * 128 + 128)
                            for nh in range(2):
                                s = slice(nh * 512, nh * 512 + 512)
                                mmb(S2[:, nh * 512:nh * 512 + 512],
                                    kt[0:64, jsl], qt[0:64, s],
                                    start=True, stop=True,
                                    tile_position=(0, 0))
                                mmb(S2[:, 1024 + nh * 512:
                                       1024 + nh * 512 + 512],
                                    kt[64:128, jsl], qt[64:128, s],
                                    start=True, stop=True,
                                    tile_position=(64, 0))
                            E2 = attt.tile([128, 2048], bf16, tag="E2",
                                           bufs=2, name="E2")
                            nc.scalar.activation(E2, S2, ACTF.Exp)
                            for hi in range(2):
                                h = hg * 2 + hi
                                P = attt.tile([128, V], bf16, tag="P",
                                              bufs=3, name="P")
                                nc.vector._custom_dve(
                                    op, out=P,
                                    in0=E2[:, hi * 1024:hi * 1024 + 1024],
                                    in1=eit[jt],
                                    s0=qc[h][0], s1=qc[h][1],
                                    imm2=qc[h][2])
                                for nh in range(2):
                                    s = slice(nh * 512, nh * 512 + 512)
                                    mmb(ops[hi][:, s], vaug[jt][:, h, :],
                                        P[:, s], start=(jt == 0),
                                        stop=(jt == 7))
                        # normalize the head pair: copy numerators to
                        # SBUF (frees PSUM), 1/den via exp(-ln(den)) on the
                        # scalar engine, broadcast+multiply on gpsimd.
                        for hi in range(2):
                            num_sb = attt.tile([64, V], f32,
                                               tag=f"num{hi}", bufs=2,
                                               name="num_sb")
                            nc.vector.tensor_copy(out=num_sb,
                                                  in_=ops[hi][0:64, :])
                            dln = attt.tile([1, V], f32, tag=f"dln{hi}",
                                            bufs=2, name="dln")
                            nc.scalar.activation(dln, ops[hi][64:65, :],
                                                 ACTF.Ln)
                            rl = attt.tile([1, V], f32, tag=f"rl{hi}",
                                           bufs=2, name="rl")
                            nc.scalar.activation(rl, dln, ACTF.Exp,
                                                 scale=-1.0)
                            rlb = attt.tile([64, V], f32, tag=f"rlb{hi}",
                                            bufs=2, name="rlb")
                            nc.gpsimd.partition_broadcast(rlb, rl)
                            nc.gpsimd.tensor_mul(
                                att[hg][hi * 64:hi * 64 + 64, :],
                                num_sb, rlb)
                # proj + residual -> x2
                with tc.tile_pool(name="projp", bufs=4,
                                  space="PSUM") as projp:
                    for m in range(4):
                        for nh in range(2):
                            s = slice(nh * 512, nh * 512 + 512)
                            pp = projp.tile([128, 512], f32, tag="mmproj")
                            for kc in range(4):
                                mmb(pp, wp_t[kc][:, m * 128:(m + 1) * 128],
                                    att[kc][:, s], start=(kc == 0),
                                    stop=(kc == 3))
                            nc.vector.scalar_tensor_tensor(
                                x2[m][:, s], pp, bp_t[:, m:m + 1],
                                xT_t[m][:, s].bitcast(f32), ALU.add,
                                ALU.add)

            # ---- MLP branch ----
            with tc.tile_pool(name="mlplife", bufs=1) as mlplife:
                h2 = adaln(x2, mlplife, "h2", x2[0])
                with tc.tile_pool(name="mlpt", bufs=1) as mlpt, \
                        tc.tile_pool(name="mlpp", bufs=4,
                                     space="PSUM") as mlpp:
                    for nh in range(2):
                        s = slice(nh * 512, nh * 512 + 512)
                        g = [mlpt.tile([128, 512], bf16, tag=f"g{m}",
                                       name=f"g{m}") for m in range(16)]
                        for m in range(16):
                            pp = mlpp.tile([128, 512], f32, tag="mmm1")
                            for kc in range(4):
                                mmb(pp, wm1_t[kc][:, m * 128:(m + 1) * 128],
                                    h2[kc][:, s], start=(kc == 0),
                                    stop=(kc == 3))
                            nc.scalar.activation(g[m], pp, ACTF.Gelu,
                                                 bias=bm1_t[:, m:m + 1])
                        for m in range(4):
                            pp = mlpp.tile([128, 512], f32, tag="mmm2")
                            for kc in range(16):
                                mmb(pp, wm2_t[kc][:, m * 128:(m + 1) * 128],
                                    g[kc], start=(kc == 0), stop=(kc == 15))
                            yt = mlpt.tile([128, 512], f32, tag="yt",
                                           bufs=2, name="yt")
                            nc.vector.scalar_tensor_tensor(
                                yt, pp, bm2_t[:, m:m + 1],
                                x2[m][:, s].bitcast(f32), ALU.add, ALU.add)
                            nc.sync.dma_start(
                                out=yT[m * 128:(m + 1) * 128, s], in_=yt)

    nc.compile()
    return nc


def _poly_coeffs(edge_table):
    """Per-head cubic p(e) = 1 + e*(q0 + e*(q1 + e*q2)) with
    p(e) = exp(table[e,h] - table[0,h]) for e = 0..3."""
    et = np.asarray(edge_table, dtype=np.float64)
    A = np.array([[1.0, 1.0, 1.0],
                  [1.0, 2.0, 4.0],
                  [1.0, 3.0, 9.0]])
    qc = []
    for h in range(H):
        g = np.exp(et[:, h] - et[0, h])
        rhs = np.array([(g[1] - 1.0) / 1.0,
                        (g[2] - 1.0) / 2.0,
                        (g[3] - 1.0) / 3.0])
        q = np.linalg.solve(A, rhs)
        # verify interpolation
        e = np.arange(4.0)
        p = 1.0 + e * (q[0] + e * (q[1] + e * q[2]))
        assert np.abs(p - g).max() < 1e-9
        qc.append([float(v) for v in q])
    return qc


def _silu(v):
    return v / (1.0 + np.exp(-v))


def _make_in_maps(inputs):
    x = np.asarray(inputs["x"], dtype=np.float32)
    cond = np.asarray(inputs["cond"], dtype=np.float32)
    ei = np.asarray(inputs["edge_index"])
    w_qkv = np.asarray(inputs["w_qkv"], dtype=np.float32)

    scale = 1.0 / np.sqrt(HD)
    wqk = w_qkv[:, :2 * D].copy()
    wqk[:, :D] *= scale
    wv = np.ascontiguousarray(w_qkv[:, 2 * D:])
    wm1 = np.asarray(inputs["w_mlp1"], dtype=np.float32)
    bm1 = np.asarray(inputs["b_mlp1"], dtype=np.float32)

    # host-side AdaLN parameter path; the scale folds into the consumer
    # weights, the shift into their (per-partition) output biases.
    sc = _silu(cond)  # [B, Dc]
    p1 = sc @ np.asarray(inputs["w_ada1"], dtype=np.float32) \
        + np.asarray(inputs["b_ada1"], dtype=np.float32)  # [B, 2D]
    p2 = sc @ np.asarray(inputs["w_ada2"], dtype=np.float32) \
        + np.asarray(inputs["b_ada2"], dtype=np.float32)
    s1, t1 = 1.0 + p1[:, :D], p1[:, D:]  # [B, D] each
    s2, t2 = 1.0 + p2[:, :D], p2[:, D:]

    shared = {
        "onesf": np.ones((128, 1), dtype=np.float32),
        "onesb": np.ones((128, 8), dtype=ml_dtypes.bfloat16),
        "wproj": np.asarray(inputs["w_proj"],
                            dtype=np.float32).astype(ml_dtypes.bfloat16),
        "wm2": np.asarray(inputs["w_mlp2"],
                          dtype=np.float32).astype(ml_dtypes.bfloat16),
        "bprojd": np.ascontiguousarray(
            np.asarray(inputs["b_proj"],
                       dtype=np.float32).reshape(4, 128).T),
        "bm2d": np.ascontiguousarray(
            np.asarray(inputs["b_mlp2"],
                       dtype=np.float32).reshape(4, 128).T),
    }
    in_maps = []
    for b in range(B):
        cqk = wqk.T @ t1[b]                      # [1024]
        cv = wv.T @ t1[b]                        # [512]
        cvb = np.zeros((128, 8, 65), dtype=ml_dtypes.bfloat16)
        cvb[:, :, 0:64] = cv.reshape(8, 64).astype(ml_dtypes.bfloat16)
        cvb[:, :, 64] = ml_dtypes.bfloat16(1.0)
        bm1_b = bm1 + wm1.T @ t2[b]              # [2048]
        in_maps.append(dict(
            shared,
            xT=np.ascontiguousarray(x[b].T),
            eiT=np.ascontiguousarray(ei[b].T.astype(ml_dtypes.bfloat16)),
            wqk=(wqk * s1[b][:, None]).astype(ml_dtypes.bfloat16),
            wv=(wv * s1[b][:, None]).astype(ml_dtypes.bfloat16),
            wm1=(wm1 * s2[b][:, None]).astype(ml_dtypes.bfloat16),
            cqkd=np.ascontiguousarray(cqk.reshape(8, 128).T),
            cvbd=np.ascontiguousarray(cvb.reshape(128, 520)),
            bm1d=np.ascontiguousarray(bm1_b.reshape(16, 128).T),
        ))
    return in_maps


def kernel(**inputs):
    from concourse.bass_utils import run_bass_kernel_spmd

    et = np.asarray(inputs["edge_table"], dtype=np.float32)
    qc = _poly_coeffs(et)

    key = (et.tobytes(),)
    if key not in _cache:
        _cache[key] = _build_program(qc)
    nc = _cache[key]

    in_maps = _make_in_maps(inputs)
    res = run_bass_kernel_spmd(nc, in_maps, core_ids=list(range(NCORES)))
    out = np.stack([np.ascontiguousarray(res.results[b]["yT"].T)
                    for b in range(B)])
    return out.astype(np.float32)
